# revision 16
# baseline (speedup 1.0000x reference)
"""AdaptiveAngleConv Trainium2 kernel — 8-core data-parallel Bass/Tile.

Per-sample dynamic 3x3 conv (256->256ch, 80x80) with attention-synthesized
weights (moe-style routing over 5 rotated kernel variants). Batch 16 is
sharded 2 samples/core across 8 NeuronCores; no collectives needed.

Device-side pipeline per core (all compute on device):
  1. x is DMA'd in fp16 row-chunks into SBUF with one zero row of vertical
     padding top/bottom (horizontal padding is virtual: edge taps use
     column-windowed PSUM writes, center tap first for has_written).
     Chunk DMAs are dependency-chained (cin-block 1 after block 0, sample
     b after b-1) so the earliest-needed bytes get full HBM bandwidth,
     and PE warm-up matmuls are paced by the chunk deps to hold the HAM
     clock gate open through the DMA phase.
  2. Global-avg-pool runs as per-chunk DVE reduces overlapped with DMA.
  3. The attention head (relu MLP, softmax over 5 angles, sigmoids) runs
     in fp32 on 1-16 partitions; softmax normalization rides in the lhsT
     of the broadcast matmul that replicates the row across partitions.
  4. Weight synthesis runs on TensorE: diag(n_att_k * cin_att) matmuls
     against ring-ordered base-kernel slices accumulate the rotation mix
     in PSUM (rotating a 3x3 kernel = shifting its 8-tap ring, so each
     rotation is a contiguous window; wrapped windows split in two);
     k2_att scales on PSUM evacuation into the fp16 agg weights.
  5. The conv runs as 18 accumulating matmuls per output tile (9 taps x
     2 cin blocks) at N=480/~320 free-dim, fp16 operands (LDWEIGHTS fully
     hidden), out_att scale + bias fused into the ScalarE epilogue.

Host-side work is layout-only: sharding x, fp16 casts, pre-transposing
the small weight tensors, replicating them per core.
"""

import numpy as np

import concourse.bass as bass
import concourse.mybir as mybir
import concourse.tile as tile
from concourse import bacc
from concourse.bass_utils import run_bass_kernel_spmd

# ---------------------------------------------------------------- constants
P = 128
BS, CIN, COUT, H, W = 16, 256, 256, 80, 80
HID, K, TEMP = 16, 5, 30.0
NCORES = 8
BSL = BS // NCORES            # samples per core
CB = CIN // P                 # cin partition blocks
OC = COUT // P                # cout partition blocks
R_TILE = 5                    # output rows per psum tile (5*80=400 <= 512)

# clockwise ring order of the 8 non-center taps of a 3x3 kernel (flat idx)
RING = [0, 1, 2, 5, 8, 7, 6, 3]
SHIFTS = [0, 1, 2, 3, 4]      # ring shifts for angles 0/45/90/135/180

F32 = mybir.dt.float32
F32R = mybir.dt.float32r
F16 = mybir.dt.float16

AF = mybir.ActivationFunctionType
ALU = mybir.AluOpType
AX = mybir.AxisListType


# ---------------------------------------------------------------- builder
def _tile_rows(h):
    """Output-row tiling: 6-row tiles (N=480) with a 4+4 tail when needed."""
    rows, r0 = [], 0
    while h - r0 >= 6:
        if h - r0 == 8:
            break
        rows.append((r0, 6))
        r0 += 6
    while h - r0 > 0:
        rows.append((r0, 4))
        r0 += 4
    assert sum(r for _, r in rows) == h
    return rows


def _emit(tc, aps, dt_mm, h, w, bsl):
    nc = tc.nc
    hp, wp = h + 2, w + 2
    tiles = _tile_rows(h)
    GROUP = 5          # tiles per conv group; cps has 6 banks so one slot
    # is always spare and the next group never stalls on epilogue lag
    NCHUNK = 8 if h % 8 == 0 else 4
    ch = h // NCHUNK
    assert h % NCHUNK == 0

    (x_d, wring_d, attcat_d, wnetT_d, wcinT_d, woutT_d, bsumT_d,
     ident_d, ones_d, zeros_d, out_d) = aps

    DT = dt_mm  # dtype for matmul operand tiles (F16, F32R or F32)
    # F32R data is bit-compatible with F32; engines that read it as plain
    # f32 need a bitcast. F16/F32 are read natively.
    xf = (lambda ap: ap.bitcast(F32)) if dt_mm == F32R else (lambda ap: ap)

    def mm(out, lhsT, rhs, start, stop):
        nc.tensor.matmul(out, lhsT, rhs, start=start, stop=stop)

    import contextlib
    from concourse.tile_rust import add_dep_helper
    with contextlib.ExitStack() as ctx:
        persist = ctx.enter_context(tc.tile_pool(name="persist", bufs=1))
        diagp = ctx.enter_context(tc.tile_pool(name="diagp", bufs=6))
        stagep = ctx.enter_context(tc.tile_pool(name="stagep", bufs=4))
        cps = ctx.enter_context(tc.tile_pool(name="cps", bufs=GROUP + 1, space="PSUM"))
        sps = ctx.enter_context(tc.tile_pool(name="sps", bufs=2, space="PSUM"))

        # ------------------------------------------------ persistent tiles
        wring = [persist.tile([P, 9, COUT], DT, name=f"wring{cb}")
                 for cb in range(CB)]
        wnetT = [persist.tile([P, HID], F32, name=f"wnetT{cb}")
                 for cb in range(CB)]
        attcat = persist.tile([HID, K + 9], F32, name="attcat")
        wcinT = persist.tile([HID, CIN], F32, name="wcinT")
        woutT = persist.tile([HID, COUT], F32, name="woutT")
        bsumT = [persist.tile([P, K], F32, name=f"bsumT{o}") for o in range(OC)]
        ident = persist.tile([P, P], DT, name="ident")
        ones = persist.tile([1, P], F32, name="ones")
        ones_s = persist.tile([1, P], F32, name="ones_s")
        zeros = persist.tile([P, max(h, w) + 2], DT, name="zeros")
        xp = [[persist.tile([P, hp, w], DT, name=f"xp{b}_{cb}")
               for cb in range(CB)] for b in range(bsl)]
        agg = [[persist.tile([P, 9, COUT], DT, name=f"agg{b}_{cb}")
                for cb in range(CB)] for b in range(bsl)]
        pooled = [persist.tile([P, bsl], F32, name=f"pooled{cb}")
                  for cb in range(CB)]
        partials = [persist.tile([P, NCHUNK], F32, name=f"part{cb}")
                    for cb in range(CB)]
        cin_att = [persist.tile([P, bsl], F32, name=f"cina{cb}")
                   for cb in range(CB)]
        out_att = [persist.tile([P, bsl], F32, name=f"outa{o}")
                   for o in range(OC)]
        aggb = [persist.tile([P, bsl], F32, name=f"aggb{o}") for o in range(OC)]
        natt = [persist.tile([P, K + 9], F32, name=f"natt{b}")
                for b in range(bsl)]
        hsb = [persist.tile([HID, 1], F32, name=f"h{b}") for b in range(bsl)]
        rowsb = [persist.tile([1, K + 9], F32, name=f"row{b}")
                 for b in range(bsl)]
        mx = persist.tile([1, 2], F32, name="mx")
        warm_ps = cps.tile([P, P], F32, tag="c", name="warm_ps")

        # ------------------------------------------------ early tiny consts
        nc.sync.dma_start(zeros[:], zeros_d[:])
        nc.sync.dma_start(ident[:], ident_d[:])

        def emit_bulk_consts():
            for cb in range(CB):
                nc.sync.dma_start(wring[cb][:], wring_d[cb])
                nc.sync.dma_start(wnetT[cb][:], wnetT_d[cb])
            nc.sync.dma_start(attcat[:], attcat_d[:])
            nc.sync.dma_start(wcinT[:], wcinT_d[:])
            nc.sync.dma_start(woutT[:], woutT_d[:])
            for o in range(OC):
                nc.sync.dma_start(bsumT[o][:], bsumT_d[o])
            nc.sync.dma_start(ones[:], ones_d[:])

        # ------------------------------------------------ per-sample chains
        # x chunks are chained pairwise (cb1 after cb0, sample b after b-1)
        # so the earliest-needed data gets the full DMA bandwidth while all
        # chunks of one block still spread across queues.
        prev_chunks = None
        for b in range(bsl):
            # ---- x load + pad, in row chunks (pooling overlaps DMA)
            for cb in range(CB):
                t = xp[b][cb]
                nc.sync.dma_start(t[:, 0, :], zeros[:, 0:w])
                nc.sync.dma_start(t[:, hp - 1, :], zeros[:, 0:w])
                chunk_insts = []
                for c in range(NCHUNK):
                    d = nc.sync.dma_start(
                        t[:, 1 + c * ch:1 + (c + 1) * ch, :],
                        x_d[b, cb * P:(cb + 1) * P, c * ch:(c + 1) * ch, :])
                    if prev_chunks is not None:
                        add_dep_helper(d.ins, prev_chunks[c],
                                       reason="pipeline x DMA blocks")
                    chunk_insts.append(d.ins)
                    if b == 0:
                        # PE warm-up matmuls paced by the x DMA stream so the
                        # HAM clock gate is released before real work starts
                        for wi in range(5):
                            wmm = nc.tensor.matmul(
                                warm_ps[:], ident[:], ident[:],
                                start=True, stop=True)
                            add_dep_helper(wmm.ins, d.ins,
                                           reason="pace warmup with DMA")
                prev_chunks = chunk_insts
            if b == 0:
                emit_bulk_consts()

            # ---- pooling (chunk partials as DMA lands, then combine)
            for cb in range(CB):
                for c in range(NCHUNK):
                    nc.vector.tensor_reduce(
                        partials[cb][:, c:c + 1],
                        xf(xp[b][cb][:, 1 + c * ch:1 + (c + 1) * ch, :]),
                        axis=AX.XY, op=ALU.add)
                nc.vector.tensor_reduce(pooled[cb][:, b:b + 1], partials[cb][:],
                                        axis=AX.X, op=ALU.add)

            # ---- attention
            ph = sps.tile([HID, 1], F32, tag="s", name=f"ph{b}")
            for cb in range(CB):
                nc.tensor.matmul(ph[:], wnetT[cb][:], pooled[cb][:, b:b + 1],
                                 start=(cb == 0), stop=(cb == CB - 1))
            nc.scalar.activation(hsb[b][:], ph[:], AF.Relu, scale=1.0 / (h * w))

            prow = sps.tile([1, K + 9], F32, tag="s", name=f"prow{b}")
            nc.tensor.matmul(prow[:], hsb[b][:], attcat[:], start=True,
                             stop=True)
            # logits/TEMP are tiny, so exp without max-subtraction is safe;
            # accum_out gives the softmax denominator in the same op. The
            # 1/sum normalization rides in the broadcast matmul's lhsT.
            nc.scalar.activation(rowsb[b][:, 0:K], prow[:, 0:K], AF.Exp,
                                 scale=1.0 / TEMP, accum_out=mx[:, 0:1])
            nc.vector.reciprocal(mx[:, 1:2], mx[:, 0:1])
            nc.vector.tensor_scalar(ones_s[:], ones[:], mx[:, 1:2], None,
                                    op0=ALU.mult)
            nc.scalar.activation(rowsb[b][:, K:K + 9], prow[:, K:K + 9],
                                 AF.Sigmoid)
            pb = sps.tile([P, K + 9], F32, tag="s", name=f"pb{b}")
            nc.tensor.matmul(pb[:, 0:K], ones_s[:], rowsb[b][:, 0:K],
                             start=True, stop=True)
            nc.tensor.matmul(pb[:, K:K + 9], ones[:], rowsb[b][:, K:K + 9],
                             start=True, stop=True)
            nc.vector.tensor_copy(natt[b][:], pb[:])

            for cb in range(CB):
                pc = sps.tile([P, 1], F32, tag="s", name=f"pc{b}_{cb}")
                nc.tensor.matmul(pc[:], wcinT[:, cb * P:(cb + 1) * P],
                                 hsb[b][:], start=True, stop=True)
                nc.scalar.activation(cin_att[cb][:, b:b + 1], pc[:], AF.Sigmoid)
            for o in range(OC):
                po = sps.tile([P, 1], F32, tag="s", name=f"po{b}_{o}")
                nc.tensor.matmul(po[:], woutT[:, o * P:(o + 1) * P],
                                 hsb[b][:], start=True, stop=True)
                nc.scalar.activation(out_att[o][:, b:b + 1], po[:], AF.Sigmoid)
                tmp5 = diagp.tile([P, K], F32, tag="tmp5", name=f"tmp5_{b}_{o}")
                nc.vector.tensor_tensor(tmp5[:], bsumT[o][:], natt[b][:, 0:K],
                                        op=ALU.mult)
                nc.vector.reduce_sum(aggb[o][:, b:b + 1], tmp5[:], axis=AX.X)

            # ---- weight synthesis
            for cb in range(CB):
                diags = []
                for k in range(K):
                    dg = diagp.tile([P, P], DT, tag="diag",
                                    name=f"dg{b}_{cb}_{k}")
                    nc.vector.tensor_scalar(dg[:], xf(ident[:]),
                                            natt[b][:, k:k + 1],
                                            cin_att[cb][:, b:b + 1],
                                            op0=ALU.mult, op1=ALU.mult)
                    diags.append(dg)
                dgc = diagp.tile([P, P], DT, tag="diag", name=f"dgc{b}_{cb}")
                nc.vector.tensor_scalar(dgc[:], xf(ident[:]),
                                        cin_att[cb][:, b:b + 1], None,
                                        op0=ALU.mult)
                psc = sps.tile([P, 512], F32, tag="s", name=f"psc{b}_{cb}")
                mm(psc[:, 0:256], dgc[:], wring[cb][:, 8, :], True, True)
                nc.vector.tensor_scalar(agg[b][cb][:, 8, :], psc[:, 0:256],
                                        natt[b][:, K + 4:K + 5], None,
                                        op0=ALU.mult)
                for j in range(4):
                    ps = sps.tile([P, 512], F32, tag="s", name=f"ps{b}_{cb}_{j}")
                    for k, s in enumerate(SHIFTS):
                        i0 = (2 * j - s) % 8
                        st_, sp_ = k == 0, k == K - 1
                        if i0 <= 6:
                            mm(ps[:], diags[k][:],
                               wring[cb][:, i0:i0 + 2, :], st_, sp_)
                        else:  # ring pair wraps 7 -> 0: two half matmuls
                            mm(ps[:, 0:256], diags[k][:],
                               wring[cb][:, 7:8, :], st_, sp_)
                            mm(ps[:, 256:512], diags[k][:],
                               wring[cb][:, 0:1, :], st_, sp_)
                    for hf in range(2):
                        q = RING[2 * j + hf]
                        nc.vector.tensor_scalar(
                            agg[b][cb][:, 2 * j + hf, :],
                            ps[:, hf * 256:(hf + 1) * 256],
                            natt[b][:, K + q:K + q + 1], None, op0=ALU.mult)

        # ------------------------------------------------ conv + epilogue
        for b in range(bsl):
            for o in range(OC):
                for g0 in range(0, len(tiles), GROUP):
                    gt = tiles[g0:g0 + GROUP]
                    pts = [cps.tile([P, rr, w], F32, tag="c",
                                    name=f"pt{b}_{o}_{g0 + i}")
                           for i, (_, rr) in enumerate(gt)]
                    slot_order = [8] + list(range(8))  # center covers the
                    # full tile and must come first (has_written semantics)
                    for si, islot in enumerate(slot_order):
                        q = RING[islot] if islot < 8 else 4
                        dy, dx = q // 3, q % 3
                        # horizontal padding is virtual: edge taps write a
                        # 79-col window of PSUM from a shifted x window
                        oc0, oc1 = (1, w) if dx == 0 else (0, w)
                        ic0, ic1 = (0, w - 1) if dx == 0 else (dx - 1, w)
                        if dx == 2:
                            oc1 = w - 1
                        for cb in range(CB):
                            lhsT = agg[b][cb][:, islot, o * P:(o + 1) * P]
                            for ti, (r0, rr) in enumerate(gt):
                                rhs = xp[b][cb][:, r0 + dy:r0 + dy + rr,
                                                ic0:ic1]
                                mm(pts[ti][:, :, oc0:oc1], lhsT, rhs,
                                   si == 0 and cb == 0,
                                   si == 8 and cb == CB - 1)
                    for ti, (r0, rr) in enumerate(gt):
                        st = stagep.tile([P, rr, w], F32, tag="stage",
                                         name=f"st{b}_{o}_{g0 + ti}")
                        nc.scalar.activation(st[:], pts[ti][:], AF.Identity,
                                             bias=aggb[o][:, b:b + 1],
                                             scale=out_att[o][:, b:b + 1])
                        nc.sync.dma_start(
                            out_d[b, o * P:(o + 1) * P, r0:r0 + rr, :], st[:])


def build_graph(dt_mm=F32R, h=H, w=W, bsl=BSL):
    nc = bacc.Bacc("TRN2", target_bir_lowering=False, debug=False,
                   num_devices=NCORES)
    aps = (
        nc.dram_tensor("x", [bsl, CIN, h, w], dt_mm, kind="ExternalInput").ap(),
        nc.dram_tensor("w_ring2", [CB, P, 9, COUT], dt_mm,
                       kind="ExternalInput").ap(),
        nc.dram_tensor("att_cat", [HID, K + 9], F32, kind="ExternalInput").ap(),
        nc.dram_tensor("w_netT", [CB, P, HID], F32, kind="ExternalInput").ap(),
        nc.dram_tensor("w_cinT", [HID, CIN], F32, kind="ExternalInput").ap(),
        nc.dram_tensor("w_outT", [HID, COUT], F32, kind="ExternalInput").ap(),
        nc.dram_tensor("b_sumT", [OC, P, K], F32, kind="ExternalInput").ap(),
        nc.dram_tensor("ident", [P, P], dt_mm, kind="ExternalInput").ap(),
        nc.dram_tensor("ones", [1, P], F32, kind="ExternalInput").ap(),
        nc.dram_tensor("zeros", [P, max(w, h) + 2], dt_mm,
                       kind="ExternalInput").ap(),
        nc.dram_tensor("out", [bsl, COUT, h, w], F32, kind="ExternalOutput").ap(),
    )
    with tile.TileContext(nc) as tc:
        _emit(tc, aps, dt_mm, h, w, bsl)
    nc.compile()
    return nc


# ---------------------------------------------------------------- host prep
def round_f32r(a):
    """Round float32 array to fp32r (8-bit exp, 11-bit mantissa, RNE)."""
    u = np.ascontiguousarray(a, dtype=np.float32).view(np.uint32)
    rem = u & np.uint32(0xFFF)
    half = np.uint32(0x800)
    lsb = (u >> np.uint32(12)) & np.uint32(1)
    add = ((rem > half) | ((rem == half) & (lsb == 1))).astype(np.uint32)
    out = ((u & np.uint32(0xFFFFF000)) + (add << np.uint32(12))).astype(np.uint32)
    return out.view(np.float32)


def _conv_mode(a, mode):
    if mode == "f32r":
        return round_f32r(a)
    if mode == "f16":
        return np.ascontiguousarray(a, dtype=np.float32).astype(np.float16)
    return np.ascontiguousarray(a, dtype=np.float32)


def prep_consts(w_base, b_base, b_extra, w_net, w_nfc, w_cin, w_k2, w_out,
                mode="f32r"):
    f = np.float32
    wflat = w_base.reshape(COUT, CIN, 9).astype(f)
    wT = np.ascontiguousarray(wflat.transpose(1, 2, 0))      # [cin, 9, cout]
    ringidx = RING + [4]                                      # 9 slots
    wring = np.ascontiguousarray(wT[:, ringidx, :])           # [cin, 9, cout]
    return {
        "w_ring2": _conv_mode(wring, mode).reshape(CB, P, 9, COUT),
        "att_cat": np.ascontiguousarray(
            np.concatenate([w_nfc.T, w_k2.T], axis=1).astype(f)),
        "w_netT": np.ascontiguousarray(w_net.T.astype(f)).reshape(CB, P, HID),
        "w_cinT": np.ascontiguousarray(w_cin.T.astype(f)),
        "w_outT": np.ascontiguousarray(w_out.T.astype(f)),
        "b_sumT": np.ascontiguousarray(
            np.concatenate([b_base[None], b_extra], axis=0).T.astype(f)
        ).reshape(OC, P, K),
        "ident": _conv_mode(np.eye(P, dtype=f), mode),
        "ones": np.ones((1, P), dtype=f),
        "zeros": _conv_mode(np.zeros((P, max(H, W) + 2), dtype=f), mode),
    }


_CACHE = {}


DT_MODE = "f16"          # "f16" | "f32r" | "f32"
_DT_OF = {"f16": F16, "f32r": F32R, "f32": F32}


def kernel(**inputs):
    mode = _CACHE.setdefault("mode", DT_MODE)
    if "nc" not in _CACHE:
        _CACHE["nc"] = build_graph(dt_mm=_DT_OF[mode])
    nc = _CACHE["nc"]
    x = _conv_mode(inputs["x"], mode)
    consts = prep_consts(
        inputs["w_base"], inputs["b_base"], inputs["b_extra"],
        inputs["w_net"], inputs["w_nfc"], inputs["w_cin"],
        inputs["w_k2"], inputs["w_out"], mode=mode)
    in_maps = [dict(consts, x=x[i * BSL:(i + 1) * BSL]) for i in range(NCORES)]
    res = run_bass_kernel_spmd(nc, in_maps, list(range(NCORES)),
                               **_CACHE.get("run_kwargs", {}))
    _CACHE["last_result"] = res
    out = np.concatenate([res.results[i]["out"] for i in range(NCORES)], axis=0)
    return out.astype(np.float32)


# revision 17
# speedup vs baseline: 1.0006x; 1.0006x over previous
"""AdaptiveAngleConv Trainium2 kernel — 8-core data-parallel Bass/Tile.

Per-sample dynamic 3x3 conv (256->256ch, 80x80) with attention-synthesized
weights (moe-style routing over 5 rotated kernel variants). Batch 16 is
sharded 2 samples/core across 8 NeuronCores; no collectives needed.

Device-side pipeline per core (all compute on device):
  1. x is DMA'd in fp16 row-chunks into SBUF with one zero row of vertical
     padding top/bottom (horizontal padding is virtual: edge taps use
     column-windowed PSUM writes, center tap first for has_written).
     Chunk DMAs are dependency-chained (cin-block 1 after block 0, sample
     b after b-1) so the earliest-needed bytes get full HBM bandwidth,
     and PE warm-up matmuls are paced by the chunk deps to hold the HAM
     clock gate open through the DMA phase.
  2. Global-avg-pool runs as per-chunk DVE reduces overlapped with DMA.
  3. The attention head (relu MLP, softmax over 5 angles, sigmoids) runs
     in fp32 on 1-16 partitions; softmax normalization rides in the lhsT
     of the broadcast matmul that replicates the row across partitions.
  4. Weight synthesis runs on TensorE: diag(n_att_k * cin_att) matmuls
     against ring-ordered base-kernel slices accumulate the rotation mix
     in PSUM (rotating a 3x3 kernel = shifting its 8-tap ring, so each
     rotation is a contiguous window; wrapped windows split in two);
     k2_att scales on PSUM evacuation into the fp16 agg weights.
  5. The conv runs as 18 accumulating matmuls per output tile (9 taps x
     2 cin blocks) at N=480/~320 free-dim, fp16 operands (LDWEIGHTS fully
     hidden), out_att scale + bias fused into the ScalarE epilogue.

Host-side work is layout-only: sharding x, fp16 casts, pre-transposing
the small weight tensors, replicating them per core.
"""

import numpy as np

import concourse.bass as bass
import concourse.mybir as mybir
import concourse.tile as tile
from concourse import bacc
from concourse.bass_utils import run_bass_kernel_spmd

# ---------------------------------------------------------------- constants
P = 128
BS, CIN, COUT, H, W = 16, 256, 256, 80, 80
HID, K, TEMP = 16, 5, 30.0
NCORES = 8
BSL = BS // NCORES            # samples per core
CB = CIN // P                 # cin partition blocks
OC = COUT // P                # cout partition blocks
R_TILE = 5                    # output rows per psum tile (5*80=400 <= 512)

# clockwise ring order of the 8 non-center taps of a 3x3 kernel (flat idx)
RING = [0, 1, 2, 5, 8, 7, 6, 3]
SHIFTS = [0, 1, 2, 3, 4]      # ring shifts for angles 0/45/90/135/180

F32 = mybir.dt.float32
F32R = mybir.dt.float32r
F16 = mybir.dt.float16

AF = mybir.ActivationFunctionType
ALU = mybir.AluOpType
AX = mybir.AxisListType


# ---------------------------------------------------------------- builder
def _tile_rows(h):
    """Output-row tiling: 6-row tiles (N=480) with a 4+4 tail when needed."""
    rows, r0 = [], 0
    while h - r0 >= 6:
        if h - r0 == 8:
            break
        rows.append((r0, 6))
        r0 += 6
    while h - r0 > 0:
        rows.append((r0, 4))
        r0 += 4
    assert sum(r for _, r in rows) == h
    return rows


def _emit(tc, aps, dt_mm, h, w, bsl):
    nc = tc.nc
    hp, wp = h + 2, w + 2
    tiles = _tile_rows(h)
    GROUP = 5          # tiles per conv group; cps has 6 banks so one slot
    # is always spare and the next group never stalls on epilogue lag
    NCHUNK = 8 if h % 8 == 0 else 4
    ch = h // NCHUNK
    assert h % NCHUNK == 0

    (x_d, wring_d, attcat_d, wnetT_d, wcinT_d, woutT_d, bsumT_d,
     ident_d, ones_d, zeros_d, out_d) = aps

    DT = dt_mm  # dtype for matmul operand tiles (F16, F32R or F32)
    # F32R data is bit-compatible with F32; engines that read it as plain
    # f32 need a bitcast. F16/F32 are read natively.
    xf = (lambda ap: ap.bitcast(F32)) if dt_mm == F32R else (lambda ap: ap)

    def mm(out, lhsT, rhs, start, stop):
        nc.tensor.matmul(out, lhsT, rhs, start=start, stop=stop)

    import contextlib
    from concourse.tile_rust import add_dep_helper
    with contextlib.ExitStack() as ctx:
        persist = ctx.enter_context(tc.tile_pool(name="persist", bufs=1))
        diagp = ctx.enter_context(tc.tile_pool(name="diagp", bufs=6))
        stagep = ctx.enter_context(tc.tile_pool(name="stagep", bufs=4))
        cps = ctx.enter_context(tc.tile_pool(name="cps", bufs=GROUP + 1, space="PSUM"))
        sps = ctx.enter_context(tc.tile_pool(name="sps", bufs=2, space="PSUM"))

        # ------------------------------------------------ persistent tiles
        wring = [persist.tile([P, 9, COUT], DT, name=f"wring{cb}")
                 for cb in range(CB)]
        wnetT = [persist.tile([P, HID], F32, name=f"wnetT{cb}")
                 for cb in range(CB)]
        attcat = persist.tile([HID, K + 9], F32, name="attcat")
        wcinT = persist.tile([HID, CIN], F32, name="wcinT")
        woutT = persist.tile([HID, COUT], F32, name="woutT")
        bsumT = [persist.tile([P, K], F32, name=f"bsumT{o}") for o in range(OC)]
        ident = persist.tile([P, P], DT, name="ident")
        ones = persist.tile([1, P], F32, name="ones")
        ones_s = persist.tile([1, P], F32, name="ones_s")
        zeros = persist.tile([P, max(h, w) + 2], DT, name="zeros")
        xp = [[persist.tile([P, hp, w], DT, name=f"xp{b}_{cb}")
               for cb in range(CB)] for b in range(bsl)]
        agg = [[persist.tile([P, 9, COUT], DT, name=f"agg{b}_{cb}")
                for cb in range(CB)] for b in range(bsl)]
        pooled = [persist.tile([P, bsl], F32, name=f"pooled{cb}")
                  for cb in range(CB)]
        partials = [persist.tile([P, NCHUNK], F32, name=f"part{cb}")
                    for cb in range(CB)]
        cin_att = [persist.tile([P, bsl], F32, name=f"cina{cb}")
                   for cb in range(CB)]
        out_att = [persist.tile([P, bsl], F32, name=f"outa{o}")
                   for o in range(OC)]
        aggb = [persist.tile([P, bsl], F32, name=f"aggb{o}") for o in range(OC)]
        natt = [persist.tile([P, K + 9], F32, name=f"natt{b}")
                for b in range(bsl)]
        hsb = [persist.tile([HID, 1], F32, name=f"h{b}") for b in range(bsl)]
        rowsb = [persist.tile([1, K + 9], F32, name=f"row{b}")
                 for b in range(bsl)]
        mx = persist.tile([1, 2], F32, name="mx")
        warm_ps = cps.tile([P, P], F32, tag="c", name="warm_ps")

        # ------------------------------------------------ early tiny consts
        nc.sync.dma_start(zeros[:], zeros_d[:])
        nc.sync.dma_start(ident[:], ident_d[:])

        def emit_bulk_consts():
            for cb in range(CB):
                nc.sync.dma_start(wring[cb][:], wring_d[cb])
                nc.sync.dma_start(wnetT[cb][:], wnetT_d[cb])
            nc.sync.dma_start(attcat[:], attcat_d[:])
            nc.sync.dma_start(wcinT[:], wcinT_d[:])
            nc.sync.dma_start(woutT[:], woutT_d[:])
            for o in range(OC):
                nc.sync.dma_start(bsumT[o][:], bsumT_d[o])
            nc.sync.dma_start(ones[:], ones_d[:])

        # ------------------------------------------------ per-sample chains
        # x chunks are chained pairwise (cb1 after cb0, sample b after b-1)
        # so the earliest-needed data gets the full DMA bandwidth while all
        # chunks of one block still spread across queues.
        prev_chunks = None
        for b in range(bsl):
            # ---- x load + pad, in row chunks (pooling overlaps DMA)
            for cb in range(CB):
                t = xp[b][cb]
                nc.sync.dma_start(t[:, 0, :], zeros[:, 0:w])
                nc.sync.dma_start(t[:, hp - 1, :], zeros[:, 0:w])
                chunk_insts = []
                for c in range(NCHUNK):
                    d = nc.sync.dma_start(
                        t[:, 1 + c * ch:1 + (c + 1) * ch, :],
                        x_d[b, cb * P:(cb + 1) * P, c * ch:(c + 1) * ch, :])
                    if prev_chunks is not None:
                        add_dep_helper(d.ins, prev_chunks[c],
                                       reason="pipeline x DMA blocks")
                    chunk_insts.append(d.ins)
                    if b == 0:
                        # PE warm-up matmuls paced by the x DMA stream so the
                        # HAM clock gate is released before real work starts
                        for wi in range(5):
                            wmm = nc.tensor.matmul(
                                warm_ps[:], ident[:], ident[:],
                                start=True, stop=True)
                            add_dep_helper(wmm.ins, d.ins,
                                           reason="pace warmup with DMA")
                prev_chunks = chunk_insts
            if b == 0:
                emit_bulk_consts()

            # ---- pooling (chunk partials as DMA lands, then combine)
            for cb in range(CB):
                for c in range(NCHUNK):
                    nc.vector.tensor_reduce(
                        partials[cb][:, c:c + 1],
                        xf(xp[b][cb][:, 1 + c * ch:1 + (c + 1) * ch, :]),
                        axis=AX.XY, op=ALU.add)
                nc.vector.tensor_reduce(pooled[cb][:, b:b + 1], partials[cb][:],
                                        axis=AX.X, op=ALU.add)

            # ---- attention
            ph = sps.tile([HID, 1], F32, tag="s", name=f"ph{b}")
            for cb in range(CB):
                nc.tensor.matmul(ph[:], wnetT[cb][:], pooled[cb][:, b:b + 1],
                                 start=(cb == 0), stop=(cb == CB - 1))
            nc.scalar.activation(hsb[b][:], ph[:], AF.Relu, scale=1.0 / (h * w))

            prow = sps.tile([1, K + 9], F32, tag="s", name=f"prow{b}")
            nc.tensor.matmul(prow[:], hsb[b][:], attcat[:], start=True,
                             stop=True)
            # logits/TEMP are tiny, so exp without max-subtraction is safe;
            # accum_out gives the softmax denominator in the same op. The
            # 1/sum normalization rides in the broadcast matmul's lhsT.
            nc.scalar.activation(rowsb[b][:, 0:K], prow[:, 0:K], AF.Exp,
                                 scale=1.0 / TEMP, accum_out=mx[:, 0:1])
            nc.vector.reciprocal(mx[:, 1:2], mx[:, 0:1])
            nc.vector.tensor_scalar(ones_s[:], ones[:], mx[:, 1:2], None,
                                    op0=ALU.mult)
            nc.scalar.activation(rowsb[b][:, K:K + 9], prow[:, K:K + 9],
                                 AF.Sigmoid)
            pb = sps.tile([P, K + 9], F32, tag="s", name=f"pb{b}")
            nc.tensor.matmul(pb[:, 0:K], ones_s[:], rowsb[b][:, 0:K],
                             start=True, stop=True)
            nc.tensor.matmul(pb[:, K:K + 9], ones[:], rowsb[b][:, K:K + 9],
                             start=True, stop=True)
            nc.vector.tensor_copy(natt[b][:], pb[:])

            for cb in range(CB):
                pc = sps.tile([P, 1], F32, tag="s", name=f"pc{b}_{cb}")
                nc.tensor.matmul(pc[:], wcinT[:, cb * P:(cb + 1) * P],
                                 hsb[b][:], start=True, stop=True)
                nc.scalar.activation(cin_att[cb][:, b:b + 1], pc[:], AF.Sigmoid)
            for o in range(OC):
                po = sps.tile([P, 1], F32, tag="s", name=f"po{b}_{o}")
                nc.tensor.matmul(po[:], woutT[:, o * P:(o + 1) * P],
                                 hsb[b][:], start=True, stop=True)
                nc.scalar.activation(out_att[o][:, b:b + 1], po[:], AF.Sigmoid)
                tmp5 = diagp.tile([P, K], F32, tag="tmp5", name=f"tmp5_{b}_{o}")
                nc.vector.tensor_tensor(tmp5[:], bsumT[o][:], natt[b][:, 0:K],
                                        op=ALU.mult)
                nc.vector.reduce_sum(aggb[o][:, b:b + 1], tmp5[:], axis=AX.X)

            # ---- weight synthesis
            for cb in range(CB):
                diags = []
                for k in range(K):
                    dg = diagp.tile([P, P], DT, tag="diag",
                                    name=f"dg{b}_{cb}_{k}")
                    nc.vector.tensor_scalar(dg[:], xf(ident[:]),
                                            natt[b][:, k:k + 1],
                                            cin_att[cb][:, b:b + 1],
                                            op0=ALU.mult, op1=ALU.mult)
                    diags.append(dg)
                dgc = diagp.tile([P, P], DT, tag="diag", name=f"dgc{b}_{cb}")
                nc.vector.tensor_scalar(dgc[:], xf(ident[:]),
                                        cin_att[cb][:, b:b + 1], None,
                                        op0=ALU.mult)
                psc = sps.tile([P, 512], F32, tag="s", name=f"psc{b}_{cb}")
                mm(psc[:, 0:256], dgc[:], wring[cb][:, 8, :], True, True)
                nc.vector.tensor_scalar(agg[b][cb][:, 8, :], psc[:, 0:256],
                                        natt[b][:, K + 4:K + 5], None,
                                        op0=ALU.mult)
                for j in range(4):
                    ps = sps.tile([P, 512], F32, tag="s", name=f"ps{b}_{cb}_{j}")
                    for k, s in enumerate(SHIFTS):
                        i0 = (2 * j - s) % 8
                        st_, sp_ = k == 0, k == K - 1
                        if i0 <= 6:
                            mm(ps[:], diags[k][:],
                               wring[cb][:, i0:i0 + 2, :], st_, sp_)
                        else:  # ring pair wraps 7 -> 0: two half matmuls
                            mm(ps[:, 0:256], diags[k][:],
                               wring[cb][:, 7:8, :], st_, sp_)
                            mm(ps[:, 256:512], diags[k][:],
                               wring[cb][:, 0:1, :], st_, sp_)
                    for hf in range(2):
                        q = RING[2 * j + hf]
                        nc.vector.tensor_scalar(
                            agg[b][cb][:, 2 * j + hf, :],
                            ps[:, hf * 256:(hf + 1) * 256],
                            natt[b][:, K + q:K + q + 1], None, op0=ALU.mult)

        # ------------------------------------------------ conv + epilogue
        for b in range(bsl):
            for o in range(OC):
                for g0 in range(0, len(tiles), GROUP):
                    gt = tiles[g0:g0 + GROUP]
                    pts = [cps.tile([P, rr, w], F32, tag="c",
                                    name=f"pt{b}_{o}_{g0 + i}")
                           for i, (_, rr) in enumerate(gt)]
                    slot_order = [8] + list(range(8))  # center covers the
                    # full tile and must come first (has_written semantics)
                    for si, islot in enumerate(slot_order):
                        q = RING[islot] if islot < 8 else 4
                        dy, dx = q // 3, q % 3
                        # horizontal padding is virtual: edge taps write a
                        # 79-col window of PSUM from a shifted x window
                        oc0, oc1 = (1, w) if dx == 0 else (0, w)
                        ic0, ic1 = (0, w - 1) if dx == 0 else (dx - 1, w)
                        if dx == 2:
                            oc1 = w - 1
                        for cb in range(CB):
                            lhsT = agg[b][cb][:, islot, o * P:(o + 1) * P]
                            for ti, (r0, rr) in enumerate(gt):
                                rhs = xp[b][cb][:, r0 + dy:r0 + dy + rr,
                                                ic0:ic1]
                                mm(pts[ti][:, :, oc0:oc1], lhsT, rhs,
                                   si == 0 and cb == 0,
                                   si == 8 and cb == CB - 1)
                    for ti, (r0, rr) in enumerate(gt):
                        st = stagep.tile([P, rr, w], F32, tag="stage",
                                         name=f"st{b}_{o}_{g0 + ti}")
                        if ti % 2 == 0:
                            nc.scalar.activation(st[:], pts[ti][:], AF.Identity,
                                                 bias=aggb[o][:, b:b + 1],
                                                 scale=out_att[o][:, b:b + 1])
                        else:
                            nc.vector.tensor_scalar(st[:], pts[ti][:],
                                                    out_att[o][:, b:b + 1],
                                                    aggb[o][:, b:b + 1],
                                                    op0=ALU.mult, op1=ALU.add)
                        nc.sync.dma_start(
                            out_d[b, o * P:(o + 1) * P, r0:r0 + rr, :], st[:])


def build_graph(dt_mm=F32R, h=H, w=W, bsl=BSL):
    nc = bacc.Bacc("TRN2", target_bir_lowering=False, debug=False,
                   num_devices=NCORES)
    aps = (
        nc.dram_tensor("x", [bsl, CIN, h, w], dt_mm, kind="ExternalInput").ap(),
        nc.dram_tensor("w_ring2", [CB, P, 9, COUT], dt_mm,
                       kind="ExternalInput").ap(),
        nc.dram_tensor("att_cat", [HID, K + 9], F32, kind="ExternalInput").ap(),
        nc.dram_tensor("w_netT", [CB, P, HID], F32, kind="ExternalInput").ap(),
        nc.dram_tensor("w_cinT", [HID, CIN], F32, kind="ExternalInput").ap(),
        nc.dram_tensor("w_outT", [HID, COUT], F32, kind="ExternalInput").ap(),
        nc.dram_tensor("b_sumT", [OC, P, K], F32, kind="ExternalInput").ap(),
        nc.dram_tensor("ident", [P, P], dt_mm, kind="ExternalInput").ap(),
        nc.dram_tensor("ones", [1, P], F32, kind="ExternalInput").ap(),
        nc.dram_tensor("zeros", [P, max(w, h) + 2], dt_mm,
                       kind="ExternalInput").ap(),
        nc.dram_tensor("out", [bsl, COUT, h, w], F32, kind="ExternalOutput").ap(),
    )
    with tile.TileContext(nc) as tc:
        _emit(tc, aps, dt_mm, h, w, bsl)
    nc.compile()
    return nc


# ---------------------------------------------------------------- host prep
def round_f32r(a):
    """Round float32 array to fp32r (8-bit exp, 11-bit mantissa, RNE)."""
    u = np.ascontiguousarray(a, dtype=np.float32).view(np.uint32)
    rem = u & np.uint32(0xFFF)
    half = np.uint32(0x800)
    lsb = (u >> np.uint32(12)) & np.uint32(1)
    add = ((rem > half) | ((rem == half) & (lsb == 1))).astype(np.uint32)
    out = ((u & np.uint32(0xFFFFF000)) + (add << np.uint32(12))).astype(np.uint32)
    return out.view(np.float32)


def _conv_mode(a, mode):
    if mode == "f32r":
        return round_f32r(a)
    if mode == "f16":
        return np.ascontiguousarray(a, dtype=np.float32).astype(np.float16)
    return np.ascontiguousarray(a, dtype=np.float32)


def prep_consts(w_base, b_base, b_extra, w_net, w_nfc, w_cin, w_k2, w_out,
                mode="f32r"):
    f = np.float32
    wflat = w_base.reshape(COUT, CIN, 9).astype(f)
    wT = np.ascontiguousarray(wflat.transpose(1, 2, 0))      # [cin, 9, cout]
    ringidx = RING + [4]                                      # 9 slots
    wring = np.ascontiguousarray(wT[:, ringidx, :])           # [cin, 9, cout]
    return {
        "w_ring2": _conv_mode(wring, mode).reshape(CB, P, 9, COUT),
        "att_cat": np.ascontiguousarray(
            np.concatenate([w_nfc.T, w_k2.T], axis=1).astype(f)),
        "w_netT": np.ascontiguousarray(w_net.T.astype(f)).reshape(CB, P, HID),
        "w_cinT": np.ascontiguousarray(w_cin.T.astype(f)),
        "w_outT": np.ascontiguousarray(w_out.T.astype(f)),
        "b_sumT": np.ascontiguousarray(
            np.concatenate([b_base[None], b_extra], axis=0).T.astype(f)
        ).reshape(OC, P, K),
        "ident": _conv_mode(np.eye(P, dtype=f), mode),
        "ones": np.ones((1, P), dtype=f),
        "zeros": _conv_mode(np.zeros((P, max(H, W) + 2), dtype=f), mode),
    }


_CACHE = {}


DT_MODE = "f16"          # "f16" | "f32r" | "f32"
_DT_OF = {"f16": F16, "f32r": F32R, "f32": F32}


def kernel(**inputs):
    mode = _CACHE.setdefault("mode", DT_MODE)
    if "nc" not in _CACHE:
        _CACHE["nc"] = build_graph(dt_mm=_DT_OF[mode])
    nc = _CACHE["nc"]
    x = _conv_mode(inputs["x"], mode)
    consts = prep_consts(
        inputs["w_base"], inputs["b_base"], inputs["b_extra"],
        inputs["w_net"], inputs["w_nfc"], inputs["w_cin"],
        inputs["w_k2"], inputs["w_out"], mode=mode)
    in_maps = [dict(consts, x=x[i * BSL:(i + 1) * BSL]) for i in range(NCORES)]
    res = run_bass_kernel_spmd(nc, in_maps, list(range(NCORES)),
                               **_CACHE.get("run_kwargs", {}))
    _CACHE["last_result"] = res
    out = np.concatenate([res.results[i]["out"] for i in range(NCORES)], axis=0)
    return out.astype(np.float32)


# revision 18
# speedup vs baseline: 1.1924x; 1.1918x over previous
"""AdaptiveAngleConv Trainium2 kernel — 8-core data-parallel Bass/Tile.

Per-sample dynamic 3x3 conv (256->256ch, 80x80) with attention-synthesized
weights (moe-style routing over 5 rotated kernel variants). Batch 16 is
sharded 2 samples/core across 8 NeuronCores; no collectives needed.

Device-side pipeline per core (all compute on device):
  1. x is DMA'd in fp16 row-chunks into SBUF with one zero row of vertical
     padding top/bottom (horizontal padding is virtual: edge taps use
     column-windowed PSUM writes, center tap first for has_written).
     Chunk DMAs are dependency-chained (cin-block 1 after block 0, sample
     b after b-1) so the earliest-needed bytes get full HBM bandwidth,
     and PE warm-up matmuls are paced by the chunk deps to hold the HAM
     clock gate open through the DMA phase.
  2. Global-avg-pool runs as per-chunk DVE reduces overlapped with DMA.
  3. The attention head (relu MLP, softmax over 5 angles, sigmoids) runs
     in fp32 on 1-16 partitions; softmax normalization rides in the lhsT
     of the broadcast matmul that replicates the row across partitions.
  4. Weight synthesis runs on TensorE: diag(n_att_k * cin_att) matmuls
     against ring-ordered base-kernel slices accumulate the rotation mix
     in PSUM (rotating a 3x3 kernel = shifting its 8-tap ring, so each
     rotation is a contiguous window; wrapped windows split in two);
     k2_att scales on PSUM evacuation into the fp16 agg weights.
  5. The conv runs as 18 accumulating matmuls per output tile (9 taps x
     2 cin blocks) at N=480/~320 free-dim, fp16 operands (LDWEIGHTS fully
     hidden), out_att scale + bias fused into the ScalarE epilogue.

Host-side work is layout-only: sharding x, fp16 casts, pre-transposing
the small weight tensors, replicating them per core.
"""

import numpy as np

import concourse.bass as bass
import concourse.mybir as mybir
import concourse.tile as tile
from concourse import bacc
from concourse.bass_utils import run_bass_kernel_spmd

# ---------------------------------------------------------------- constants
P = 128
BS, CIN, COUT, H, W = 16, 256, 256, 80, 80
HID, K, TEMP = 16, 5, 30.0
NCORES = 8
BSL = BS // NCORES            # samples per core
CB = CIN // P                 # cin partition blocks
OC = COUT // P                # cout partition blocks
R_TILE = 5                    # output rows per psum tile (5*80=400 <= 512)

# clockwise ring order of the 8 non-center taps of a 3x3 kernel (flat idx)
RING = [0, 1, 2, 5, 8, 7, 6, 3]
SHIFTS = [0, 1, 2, 3, 4]      # ring shifts for angles 0/45/90/135/180

F32 = mybir.dt.float32
F32R = mybir.dt.float32r
F16 = mybir.dt.float16

AF = mybir.ActivationFunctionType
ALU = mybir.AluOpType
AX = mybir.AxisListType


# ---------------------------------------------------------------- builder
def _tile_rows(h):
    """Output-row tiling: 6-row tiles (N=480) with a 4+4 tail when needed."""
    rows, r0 = [], 0
    while h - r0 >= 6:
        if h - r0 == 8:
            break
        rows.append((r0, 6))
        r0 += 6
    while h - r0 > 0:
        rows.append((r0, 4))
        r0 += 4
    assert sum(r for _, r in rows) == h
    return rows


def _emit(tc, aps, dt_mm, h, w, bsl):
    nc = tc.nc
    hp, wp = h + 2, w + 2
    tiles = _tile_rows(h)
    GROUP = 5          # tiles per conv group; cps has 6 banks so one slot
    # is always spare and the next group never stalls on epilogue lag
    NCHUNK = 8 if h % 8 == 0 else 4
    ch = h // NCHUNK
    assert h % NCHUNK == 0

    (x_d, wring_d, attcat_d, wnetT_d, wcinT_d, woutT_d, bsumT_d,
     ident_d, ones_d, zeros_d, out_d) = aps

    DT = dt_mm  # dtype for matmul operand tiles (F16, F32R or F32)
    # F32R data is bit-compatible with F32; engines that read it as plain
    # f32 need a bitcast. F16/F32 are read natively.
    xf = (lambda ap: ap.bitcast(F32)) if dt_mm == F32R else (lambda ap: ap)

    def mm(out, lhsT, rhs, start, stop):
        nc.tensor.matmul(out, lhsT, rhs, start=start, stop=stop)

    import contextlib
    from concourse.tile_rust import add_dep_helper
    with contextlib.ExitStack() as ctx:
        persist = ctx.enter_context(tc.tile_pool(name="persist", bufs=1))
        diagp = ctx.enter_context(tc.tile_pool(name="diagp", bufs=6))
        stagep = ctx.enter_context(tc.tile_pool(name="stagep", bufs=6))
        cps = ctx.enter_context(tc.tile_pool(name="cps", bufs=GROUP + 1, space="PSUM"))
        sps = ctx.enter_context(tc.tile_pool(name="sps", bufs=2, space="PSUM"))

        # ------------------------------------------------ persistent tiles
        wring = [persist.tile([P, 9, COUT], DT, name=f"wring{cb}")
                 for cb in range(CB)]
        wnetT = [persist.tile([P, HID], F32, name=f"wnetT{cb}")
                 for cb in range(CB)]
        attcat = persist.tile([HID, K + 9], F32, name="attcat")
        wcinT = persist.tile([HID, CIN], F32, name="wcinT")
        woutT = persist.tile([HID, COUT], F32, name="woutT")
        bsumT = [persist.tile([P, K], F32, name=f"bsumT{o}") for o in range(OC)]
        ident = persist.tile([P, P], DT, name="ident")
        ones = persist.tile([1, P], F32, name="ones")
        ones_s = persist.tile([1, P], F32, name="ones_s")
        zeros = persist.tile([P, max(h, w) + 2], DT, name="zeros")
        xp = [[persist.tile([P, hp, w], DT, name=f"xp{b}_{cb}")
               for cb in range(CB)] for b in range(bsl)]
        agg = [[persist.tile([P, 9, COUT], DT, name=f"agg{b}_{cb}")
                for cb in range(CB)] for b in range(bsl)]
        pooled = [persist.tile([P, bsl], F32, name=f"pooled{cb}")
                  for cb in range(CB)]
        partials = [persist.tile([P, NCHUNK], F32, name=f"part{cb}")
                    for cb in range(CB)]
        cin_att = [persist.tile([P, bsl], F32, name=f"cina{cb}")
                   for cb in range(CB)]
        out_att = [persist.tile([P, bsl], F32, name=f"outa{o}")
                   for o in range(OC)]
        aggb = [persist.tile([P, bsl], F32, name=f"aggb{o}") for o in range(OC)]
        natt = [persist.tile([P, K + 9], F32, name=f"natt{b}")
                for b in range(bsl)]
        hsb = [persist.tile([HID, 1], F32, name=f"h{b}") for b in range(bsl)]
        rowsb = [persist.tile([1, K + 9], F32, name=f"row{b}")
                 for b in range(bsl)]
        mx = persist.tile([1, 2], F32, name="mx")
        warm_ps = cps.tile([P, P], F32, tag="c", name="warm_ps")

        # ------------------------------------------------ early tiny consts
        nc.sync.dma_start(zeros[:], zeros_d[:])
        nc.sync.dma_start(ident[:], ident_d[:])

        def emit_bulk_consts():
            for cb in range(CB):
                nc.sync.dma_start(wring[cb][:], wring_d[cb])
                nc.sync.dma_start(wnetT[cb][:], wnetT_d[cb])
            nc.sync.dma_start(attcat[:], attcat_d[:])
            nc.sync.dma_start(wcinT[:], wcinT_d[:])
            nc.sync.dma_start(woutT[:], woutT_d[:])
            for o in range(OC):
                nc.sync.dma_start(bsumT[o][:], bsumT_d[o])
            nc.sync.dma_start(ones[:], ones_d[:])

        # ------------------------------------------------ per-sample chains
        # x chunks are chained pairwise (cb1 after cb0, sample b after b-1)
        # so the earliest-needed data gets the full DMA bandwidth while all
        # chunks of one block still spread across queues.
        prev_chunks = None
        for b in range(bsl):
            # ---- x load + pad, in row chunks (pooling overlaps DMA)
            for cb in range(CB):
                t = xp[b][cb]
                nc.sync.dma_start(t[:, 0, :], zeros[:, 0:w])
                nc.sync.dma_start(t[:, hp - 1, :], zeros[:, 0:w])
                chunk_insts = []
                for c in range(NCHUNK):
                    d = nc.sync.dma_start(
                        t[:, 1 + c * ch:1 + (c + 1) * ch, :],
                        x_d[b, cb * P:(cb + 1) * P, c * ch:(c + 1) * ch, :])
                    if prev_chunks is not None:
                        add_dep_helper(d.ins, prev_chunks[c],
                                       reason="pipeline x DMA blocks")
                    chunk_insts.append(d.ins)
                    if b == 0:
                        # PE warm-up matmuls paced by the x DMA stream so the
                        # HAM clock gate is released before real work starts
                        for wi in range(5):
                            wmm = nc.tensor.matmul(
                                warm_ps[:], ident[:], ident[:],
                                start=True, stop=True)
                            add_dep_helper(wmm.ins, d.ins,
                                           reason="pace warmup with DMA")
                prev_chunks = chunk_insts
            if b == 0:
                emit_bulk_consts()

            # ---- pooling (chunk partials as DMA lands, then combine)
            for cb in range(CB):
                for c in range(NCHUNK):
                    nc.vector.tensor_reduce(
                        partials[cb][:, c:c + 1],
                        xf(xp[b][cb][:, 1 + c * ch:1 + (c + 1) * ch, :]),
                        axis=AX.XY, op=ALU.add)
                nc.vector.tensor_reduce(pooled[cb][:, b:b + 1], partials[cb][:],
                                        axis=AX.X, op=ALU.add)

            # ---- attention
            ph = sps.tile([HID, 1], F32, tag="s", name=f"ph{b}")
            for cb in range(CB):
                nc.tensor.matmul(ph[:], wnetT[cb][:], pooled[cb][:, b:b + 1],
                                 start=(cb == 0), stop=(cb == CB - 1))
            nc.scalar.activation(hsb[b][:], ph[:], AF.Relu, scale=1.0 / (h * w))

            prow = sps.tile([1, K + 9], F32, tag="s", name=f"prow{b}")
            nc.tensor.matmul(prow[:], hsb[b][:], attcat[:], start=True,
                             stop=True)
            # logits/TEMP are tiny, so exp without max-subtraction is safe;
            # accum_out gives the softmax denominator in the same op. The
            # 1/sum normalization rides in the broadcast matmul's lhsT.
            nc.scalar.activation(rowsb[b][:, 0:K], prow[:, 0:K], AF.Exp,
                                 scale=1.0 / TEMP, accum_out=mx[:, 0:1])
            nc.vector.reciprocal(mx[:, 1:2], mx[:, 0:1])
            nc.vector.tensor_scalar(ones_s[:], ones[:], mx[:, 1:2], None,
                                    op0=ALU.mult)
            nc.scalar.activation(rowsb[b][:, K:K + 9], prow[:, K:K + 9],
                                 AF.Sigmoid)
            pb = sps.tile([P, K + 9], F32, tag="s", name=f"pb{b}")
            nc.tensor.matmul(pb[:, 0:K], ones_s[:], rowsb[b][:, 0:K],
                             start=True, stop=True)
            nc.tensor.matmul(pb[:, K:K + 9], ones[:], rowsb[b][:, K:K + 9],
                             start=True, stop=True)
            nc.vector.tensor_copy(natt[b][:], pb[:])

            for cb in range(CB):
                pc = sps.tile([P, 1], F32, tag="s", name=f"pc{b}_{cb}")
                nc.tensor.matmul(pc[:], wcinT[:, cb * P:(cb + 1) * P],
                                 hsb[b][:], start=True, stop=True)
                nc.scalar.activation(cin_att[cb][:, b:b + 1], pc[:], AF.Sigmoid)
            for o in range(OC):
                po = sps.tile([P, 1], F32, tag="s", name=f"po{b}_{o}")
                nc.tensor.matmul(po[:], woutT[:, o * P:(o + 1) * P],
                                 hsb[b][:], start=True, stop=True)
                nc.scalar.activation(out_att[o][:, b:b + 1], po[:], AF.Sigmoid)
                tmp5 = diagp.tile([P, K], F32, tag="tmp5", name=f"tmp5_{b}_{o}")
                nc.vector.tensor_tensor(tmp5[:], bsumT[o][:], natt[b][:, 0:K],
                                        op=ALU.mult)
                nc.vector.reduce_sum(aggb[o][:, b:b + 1], tmp5[:], axis=AX.X)

            # ---- weight synthesis
            for cb in range(CB):
                diags = []
                for k in range(K):
                    dg = diagp.tile([P, P], DT, tag="diag",
                                    name=f"dg{b}_{cb}_{k}")
                    nc.vector.tensor_scalar(dg[:], xf(ident[:]),
                                            natt[b][:, k:k + 1],
                                            cin_att[cb][:, b:b + 1],
                                            op0=ALU.mult, op1=ALU.mult)
                    diags.append(dg)
                dgc = diagp.tile([P, P], DT, tag="diag", name=f"dgc{b}_{cb}")
                nc.vector.tensor_scalar(dgc[:], xf(ident[:]),
                                        cin_att[cb][:, b:b + 1], None,
                                        op0=ALU.mult)
                psc = sps.tile([P, 512], F32, tag="s", name=f"psc{b}_{cb}")
                mm(psc[:, 0:256], dgc[:], wring[cb][:, 8, :], True, True)
                nc.vector.tensor_scalar(agg[b][cb][:, 8, :], psc[:, 0:256],
                                        natt[b][:, K + 4:K + 5], None,
                                        op0=ALU.mult)
                for j in range(4):
                    ps = sps.tile([P, 512], F32, tag="s", name=f"ps{b}_{cb}_{j}")
                    for k, s in enumerate(SHIFTS):
                        i0 = (2 * j - s) % 8
                        st_, sp_ = k == 0, k == K - 1
                        if i0 <= 6:
                            mm(ps[:], diags[k][:],
                               wring[cb][:, i0:i0 + 2, :], st_, sp_)
                        else:  # ring pair wraps 7 -> 0: two half matmuls
                            mm(ps[:, 0:256], diags[k][:],
                               wring[cb][:, 7:8, :], st_, sp_)
                            mm(ps[:, 256:512], diags[k][:],
                               wring[cb][:, 0:1, :], st_, sp_)
                    for hf in range(2):
                        q = RING[2 * j + hf]
                        nc.vector.tensor_scalar(
                            agg[b][cb][:, 2 * j + hf, :],
                            ps[:, hf * 256:(hf + 1) * 256],
                            natt[b][:, K + q:K + q + 1], None, op0=ALU.mult)

        # ------------------------------------------------ conv + epilogue
        for b in range(bsl):
            for o in range(OC):
                for g0 in range(0, len(tiles), GROUP):
                    gt = tiles[g0:g0 + GROUP]
                    pts = [cps.tile([P, rr, w], F32, tag="c",
                                    name=f"pt{b}_{o}_{g0 + i}")
                           for i, (_, rr) in enumerate(gt)]
                    slot_order = [8] + list(range(8))  # center covers the
                    # full tile and must come first (has_written semantics)
                    for si, islot in enumerate(slot_order):
                        q = RING[islot] if islot < 8 else 4
                        dy, dx = q // 3, q % 3
                        # horizontal padding is virtual: edge taps write a
                        # 79-col window of PSUM from a shifted x window
                        oc0, oc1 = (1, w) if dx == 0 else (0, w)
                        ic0, ic1 = (0, w - 1) if dx == 0 else (dx - 1, w)
                        if dx == 2:
                            oc1 = w - 1
                        for cb in range(CB):
                            lhsT = agg[b][cb][:, islot, o * P:(o + 1) * P]
                            for ti, (r0, rr) in enumerate(gt):
                                rhs = xp[b][cb][:, r0 + dy:r0 + dy + rr,
                                                ic0:ic1]
                                mm(pts[ti][:, :, oc0:oc1], lhsT, rhs,
                                   si == 0 and cb == 0,
                                   si == 8 and cb == CB - 1)
                    for ti, (r0, rr) in enumerate(gt):
                        st = stagep.tile([P, rr, w], F32, tag="stage",
                                         name=f"st{b}_{o}_{g0 + ti}")
                        if ti % 2 == 0:
                            nc.scalar.activation(st[:], pts[ti][:], AF.Identity,
                                                 bias=aggb[o][:, b:b + 1],
                                                 scale=out_att[o][:, b:b + 1])
                        else:
                            nc.vector.tensor_scalar(st[:], pts[ti][:],
                                                    out_att[o][:, b:b + 1],
                                                    aggb[o][:, b:b + 1],
                                                    op0=ALU.mult, op1=ALU.add)
                        nc.sync.dma_start(
                            out_d[b, o * P:(o + 1) * P, r0:r0 + rr, :], st[:])


def build_graph(dt_mm=F32R, h=H, w=W, bsl=BSL):
    nc = bacc.Bacc("TRN2", target_bir_lowering=False, debug=False,
                   num_devices=NCORES)
    aps = (
        nc.dram_tensor("x", [bsl, CIN, h, w], dt_mm, kind="ExternalInput").ap(),
        nc.dram_tensor("w_ring2", [CB, P, 9, COUT], dt_mm,
                       kind="ExternalInput").ap(),
        nc.dram_tensor("att_cat", [HID, K + 9], F32, kind="ExternalInput").ap(),
        nc.dram_tensor("w_netT", [CB, P, HID], F32, kind="ExternalInput").ap(),
        nc.dram_tensor("w_cinT", [HID, CIN], F32, kind="ExternalInput").ap(),
        nc.dram_tensor("w_outT", [HID, COUT], F32, kind="ExternalInput").ap(),
        nc.dram_tensor("b_sumT", [OC, P, K], F32, kind="ExternalInput").ap(),
        nc.dram_tensor("ident", [P, P], dt_mm, kind="ExternalInput").ap(),
        nc.dram_tensor("ones", [1, P], F32, kind="ExternalInput").ap(),
        nc.dram_tensor("zeros", [P, max(w, h) + 2], dt_mm,
                       kind="ExternalInput").ap(),
        nc.dram_tensor("out", [bsl, COUT, h, w], F32, kind="ExternalOutput").ap(),
    )
    with tile.TileContext(nc) as tc:
        _emit(tc, aps, dt_mm, h, w, bsl)
    nc.compile()
    return nc


# ---------------------------------------------------------------- host prep
def round_f32r(a):
    """Round float32 array to fp32r (8-bit exp, 11-bit mantissa, RNE)."""
    u = np.ascontiguousarray(a, dtype=np.float32).view(np.uint32)
    rem = u & np.uint32(0xFFF)
    half = np.uint32(0x800)
    lsb = (u >> np.uint32(12)) & np.uint32(1)
    add = ((rem > half) | ((rem == half) & (lsb == 1))).astype(np.uint32)
    out = ((u & np.uint32(0xFFFFF000)) + (add << np.uint32(12))).astype(np.uint32)
    return out.view(np.float32)


def _conv_mode(a, mode):
    if mode == "f32r":
        return round_f32r(a)
    if mode == "f16":
        return np.ascontiguousarray(a, dtype=np.float32).astype(np.float16)
    return np.ascontiguousarray(a, dtype=np.float32)


def prep_consts(w_base, b_base, b_extra, w_net, w_nfc, w_cin, w_k2, w_out,
                mode="f32r"):
    f = np.float32
    wflat = w_base.reshape(COUT, CIN, 9).astype(f)
    wT = np.ascontiguousarray(wflat.transpose(1, 2, 0))      # [cin, 9, cout]
    ringidx = RING + [4]                                      # 9 slots
    wring = np.ascontiguousarray(wT[:, ringidx, :])           # [cin, 9, cout]
    return {
        "w_ring2": _conv_mode(wring, mode).reshape(CB, P, 9, COUT),
        "att_cat": np.ascontiguousarray(
            np.concatenate([w_nfc.T, w_k2.T], axis=1).astype(f)),
        "w_netT": np.ascontiguousarray(w_net.T.astype(f)).reshape(CB, P, HID),
        "w_cinT": np.ascontiguousarray(w_cin.T.astype(f)),
        "w_outT": np.ascontiguousarray(w_out.T.astype(f)),
        "b_sumT": np.ascontiguousarray(
            np.concatenate([b_base[None], b_extra], axis=0).T.astype(f)
        ).reshape(OC, P, K),
        "ident": _conv_mode(np.eye(P, dtype=f), mode),
        "ones": np.ones((1, P), dtype=f),
        "zeros": _conv_mode(np.zeros((P, max(H, W) + 2), dtype=f), mode),
    }


_CACHE = {}


DT_MODE = "f16"          # "f16" | "f32r" | "f32"
_DT_OF = {"f16": F16, "f32r": F32R, "f32": F32}


def kernel(**inputs):
    mode = _CACHE.setdefault("mode", DT_MODE)
    if "nc" not in _CACHE:
        _CACHE["nc"] = build_graph(dt_mm=_DT_OF[mode])
    nc = _CACHE["nc"]
    x = _conv_mode(inputs["x"], mode)
    consts = prep_consts(
        inputs["w_base"], inputs["b_base"], inputs["b_extra"],
        inputs["w_net"], inputs["w_nfc"], inputs["w_cin"],
        inputs["w_k2"], inputs["w_out"], mode=mode)
    in_maps = [dict(consts, x=x[i * BSL:(i + 1) * BSL]) for i in range(NCORES)]
    res = run_bass_kernel_spmd(nc, in_maps, list(range(NCORES)),
                               **_CACHE.get("run_kwargs", {}))
    _CACHE["last_result"] = res
    out = np.concatenate([res.results[i]["out"] for i in range(NCORES)], axis=0)
    return out.astype(np.float32)
